# revision 1
# baseline (speedup 1.0000x reference)
"""Trainium2 Bass kernel for DifferentiableSparseHypergraph (topk_masking).

Full computation per batch n:
  x_mean = x[n].mean(T)                      (C, V)
  q = Wq @ x_mean + bq                       (O=32, V)   [1x1 conv == matmul]
  q = q / max(||q||_2 over O, eps)
  H_raw = (q^T @ key_prototypes) / sqrt(O)   (V, M=128)
  topk10 -> softmax over the 10 vals -> scatter back; zeros elsewhere.

Kernel strategy (pure data-parallel over batch, 8 cores x 8 batches):
  * t-mean: ONE DVE add level pairs t's (64 -> 32), then the remaining
    32-way t-sum rides the PE: 8 fp32 matmuls (4 free-chunks x 2 c-halves)
    accumulate into a single [32, 512] PSUM tile; slot (tl, v) collects
    t-pairs {tl, 8+tl, 16+tl, 24+tl}. A strided DVE reduce (8 -> 1) and an
    ACT bias/scale step produce q. This moves ~45 us of fp32 tensor_tensor
    work (1x-rate on DVE) onto the otherwise-idle PE.
  * software pipelining: the reduce/bias/score for batch n-1 are EMITTED
    after batch n's adds. DVE executes in order, so putting the reduce
    (which waits ~7 us for the PE matmul group) before the next batch's
    adds would stall the whole add stream behind it.
  * top-10 runs scale-invariantly on the RAW score matmul pb (still in
    PSUM): per-row ordering of H = pb * rn (rn > 0) equals ordering of pb,
    so max/match_replace/max finds the 10th-largest threshold t_k without
    waiting for the norm. exp(H) fuses the rn scale into ACT's Exp; the
    output is exp * (pb >= t_k) / sum -- identical to softmax-over-topk
    scattered back.
  * output DMAs issue from the ACT engine's DGE queue (ot is computed on
    ACT, so the issue is same-engine in-order, no sem) -- an out DMA on
    the sync queue head-of-line-blocks every later x-load issue.
  * batches 0 and 7 stream in t-range chunks so compute starts right as
    the first MiB lands and the tail after the last byte stays short.
"""

import numpy as np

import concourse.bacc as bacc
import concourse.bass as bass
import concourse.mybir as mybir
import concourse.tile as tile

N, C, T, V = 64, 256, 64, 64
INTER = 32          # conv out channels
M = 128             # num hyperedges
TOPK = 10
NCORES = 8
FP = mybir.dt.float32
NEG_BIG = -1.0e30


def build_nc(nloc: int) -> bass.Bass:
    """Build the per-core Bass program processing `nloc` batches."""
    assert nloc % 2 == 0
    # Bacc (not bare Bass): its compile()/finalize() pipeline splits
    # multi-semaphore waits into InstEventSemaphore pairs — walrus allows
    # at most one sync wait per regular instruction.
    nc = bacc.Bacc(target_bir_lowering=False, debug=False)

    x = nc.dram_tensor("x", (nloc, C, T, V), FP, kind="ExternalInput")
    wqt = nc.dram_tensor("wqt", (C, INTER), FP, kind="ExternalInput")
    kp = nc.dram_tensor("kp", (INTER, M), FP, kind="ExternalInput")
    bq = nc.dram_tensor("bq", (INTER, 1), FP, kind="ExternalInput")
    out = nc.dram_tensor("out", (nloc, V, M), FP, kind="ExternalOutput")

    A = mybir.AluOpType
    AF = mybir.ActivationFunctionType
    from concourse.tile import add_dep_helper

    last = nloc - 1

    with tile.TileContext(nc) as tc:
        with (
            tc.tile_pool(name="consts", bufs=1) as consts,
            tc.tile_pool(name="xph", bufs=4) as xph,
            tc.tile_pool(name="xp", bufs=2) as xp,
            tc.tile_pool(name="small", bufs=2) as small,
            tc.tile_pool(name="psA", bufs=3, space="PSUM") as psA,
            tc.tile_pool(name="psB", bufs=2, space="PSUM") as psB,
            tc.tile_pool(name="psS", bufs=1, space="PSUM") as psS,
        ):
            # --- replicated constants go first (0.13 MiB: delays batch 0 by
            # ~0.3us, but lets the PE warm-up matmuls start ~2.5us earlier)
            wq_sb = consts.tile([128, 2, INTER], FP)    # [c, c_half, o]
            nc.sync.dma_start(
                out=wq_sb[:], in_=wqt.rearrange("(h c) o -> c h o", h=2)
            )
            kp_sb = consts.tile([INTER, M], FP)
            nc.sync.dma_start(out=kp_sb[:], in_=kp[:])
            # bq / ones / 1.5 packed into one tile: each tiny tile costs a
            # 2KB-aligned SBUF slot and SBUF is full.
            cc = consts.tile([128, 4], FP)
            nc.sync.dma_start(out=cc[0:INTER, 0:1], in_=bq[:])
            bq_sb = cc[0:INTER, 0:1]

            # --- x streams into ONE [128, (h, t, v)] tile per batch: both
            # c-halves side by side in the free dim, ONE dma_start per
            # batch.  Fewer dma_starts matter: the Tile DMA pool rotates 8
            # completion sems, so issue #k waits completion #k-8 — with
            # 2-4 issues per batch the LAST batch's issues chained to
            # batch-5 completions and its data landed ~17us late.
            # Batch 0 streams in 1 MiB chunks so the DVE adds start early
            # (chunked batches pair t with t+1, whole batches t with t+32).
            # batch 0 chunks sized 1:3 — a small first chunk starts the
            # first add ~1.2us earlier, shrinking the pipeline's initial
            # phase lag (which the tick-gated issue stream inherits)
            xb0 = xph.tile([128, 2 * T * V], FP, tag="xb", name="xb0")
            for c in range(2):
                lo, hi = (0, 1024) if c == 0 else (1024, 4096)
                for h in range(2):
                    nc.sync.dma_start(
                        out=xb0[:, h * 4096 + lo : h * 4096 + hi],
                        in_=x[0, h * 128 : (h + 1) * 128, lo // V : hi // V],
                    )
            # ones-matmul scale: pc = sum_o qsq * INTER = INTER*||q||^2, so
            # rn = rsqrt(pc) directly (no separate INTER scale anywhere).
            ones_sb = cc[0:INTER, 1:2]
            nc.vector.memset(ones_sb, float(INTER))
            c15 = cc[:, 2:3]
            nc.vector.memset(c15, 1.5)

            # The fp32 self-loading matmul can carry at most ONE semaphore
            # wait (walrus S3_LW_STRUCT limit). Absorb the wq/kp DMA waits
            # with dummy 1x1 matmuls so the first real matmuls only wait on
            # their a1-tile DVE sem.
            scr = psS.tile([32, 512], FP)
            d1 = nc.tensor.matmul(
                scr[0:1, 0:1], wq_sb[:, 0, 0:1], wq_sb[:, 0, 0:1],
                start=True, stop=True,
            )
            d2 = nc.tensor.matmul(
                scr[0:1, 0:1], kp_sb[:, 0:1], kp_sb[:, 0:1],
                start=True, stop=True,
            )
            add_dep_helper(d2.ins, d1.ins, sync=False, reason="pe-wait-absorb order")

            # PE p-state warm-up: the PE clocks 0.65 -> 1.2 -> 2.4 GHz with
            # ~3us of continuous work; cold first-batch matmuls ran at
            # ~2.5x cost and that lag leaked into the DVE tick chain that
            # gates x-load issues.  Burn garbage matmuls while batch 0
            # streams in (PE is idle then anyway).
            warm = consts.tile([128, 512], FP)
            nc.gpsimd.memset(warm[:], 0.0)
            # 3, not more: with the 1:3 batch-0 chunking the first real
            # matmul is ready ~10.8us and longer warm-up runs would block it
            prev = d2
            for wi in range(3):
                wm = nc.tensor.matmul(
                    scr[:, 0:256], wq_sb[:, 0, :], warm[:, 0:256],
                    start=True, stop=True,
                )
                add_dep_helper(
                    wm.ins, prev.ins, sync=False, reason="warmup order"
                )
                prev = wm

            q2 = {}          # pair -> q2 tile
            pending = []       # [(n, pa)] awaiting reduce/bias
            ready_scores = []  # pairs whose reduce is emitted, score isn't
            first_mm = None

            def finish_reduce(n, pa):
                """Emit reduce + bias for batch n."""
                l = n % 2
                p = n // 2
                qtmp = small.tile([INTER, V], FP, tag="qtmp")
                nc.vector.reduce_sum(
                    out=qtmp[:],
                    in_=pa[:].rearrange("o (t v) -> o v t", t=8),
                    axis=mybir.AxisListType.X,
                )
                nc.scalar.activation(
                    q2[p][:, l * V : (l + 1) * V],
                    qtmp[:],
                    AF.Identity,
                    bias=bq_sb,
                    scale=1.0 / T,
                )

            def finish_score(p):
                """Emit the score/topk/softmax chain + out DMA for pair p."""
                # raw scores pb[vv, m] = q2^T . kp stay in PSUM; the top-10
                # threshold is found on pb directly (ordering-invariant to
                # the positive per-row rescale rn).
                qsq = small.tile([INTER, 2 * V], FP, tag="qsq")
                nc.scalar.activation(qsq[:], q2[p][:], AF.Square)
                pb = psB.tile([2 * V, M], FP, tag="pb")
                nc.tensor.matmul(pb[:], q2[p][:], kp_sb[:], start=True, stop=True)
                pc = psB.tile([2 * V, 1], FP, tag="pc")
                nc.tensor.matmul(pc[:], qsq[:], ones_sb, start=True, stop=True)
                # One consolidated scratch tile for all the [2V, small]
                # intermediates: separately-tagged tiny tiles each burn a
                # 2KB-aligned SBUF slot per buf and SBUF is 100% full.
                # cols: 0 k, 1 t1, 2 y0, 3 hh, 4 u1, 5 w1, 6 y1, 7 u2,
                #       8 w2, 9 rn, 10 s, 11 r, 16:24 top8a, 24:32 top8b
                sc = small.tile([2 * V, 32], FP, tag="sc")
                k, t1, y0, hh = sc[:, 0:1], sc[:, 1:2], sc[:, 2:3], sc[:, 3:4]
                u1, w1, y1 = sc[:, 4:5], sc[:, 5:6], sc[:, 6:7]
                u2, w2, rn = sc[:, 7:8], sc[:, 8:9], sc[:, 9:10]
                s, r = sc[:, 10:11], sc[:, 11:12]
                top8a, top8b = sc[:, 16:24], sc[:, 24:32]
                # rn = rsqrt(pc) on DVE (fast-inverse-sqrt + 2 Newton steps).
                # ACT's Sqrt lives in a different function table than Exp, so
                # using it costs TWO 1.28us ACT_TABLE_LOADs per pair — the
                # whole rsqrt runs on DVE in ~0.9us instead.
                U32 = mybir.dt.uint32
                I32 = mybir.dt.int32
                # read pc's bits straight from PSUM: one fewer serial DVE
                # op + dep hop on every pair's critical path
                nc.vector.tensor_scalar(
                    t1.bitcast(U32), pc[:].bitcast(U32), 1, None,
                    op0=A.logical_shift_right,
                )
                # y0bits = 0x5f3759df - t1 (DVE int "arith" rounds through
                # fp32 — ~6 low bits of seed lost, irrelevant: the magic
                # seed is only ~3% accurate anyway and Newton runs in fp32)
                nc.vector.tensor_scalar(
                    y0.bitcast(I32), t1.bitcast(I32), -1, 0x5F3759DF,
                    op0=A.mult, op1=A.add,
                )
                nc.vector.tensor_scalar(hh, pc[:], -0.5, None, op0=A.mult)
                nc.vector.tensor_mul(u1, y0, y0)
                nc.vector.scalar_tensor_tensor(
                    out=w1, in0=hh, scalar=u1, in1=c15,
                    op0=A.mult, op1=A.add,
                )
                # one Newton step suffices: rn rel err ~0.17%, which only
                # rescales exp()'s argument (|H|<~0.6 -> output err ~1e-3,
                # tolerance is 2e-2); the topk mask never sees rn.
                nc.vector.tensor_mul(rn, y0, w1)

                # t_k = 10th largest per row: top8, knock out, top8 again
                nc.vector.max(top8a, pb[:])
                work = small.tile([2 * V, M], FP, tag="work")
                nc.vector.match_replace(work[:], top8a, pb[:], NEG_BIG)
                nc.vector.max(top8b, work[:])

                # e = exp(H) = exp(pb * rn)  (rn fused into ACT's scale);
                # masked softmax without scatter:
                # me = (pb >= t_k) * e; out = me / sum(me)
                e = small.tile([2 * V, M], FP, tag="e")
                nc.scalar.activation(e[:], pb[:], AF.Exp, scale=rn)
                me = small.tile([2 * V, M], FP, tag="me")
                nc.vector.scalar_tensor_tensor(
                    out=me[:],
                    in0=pb[:],
                    scalar=sc[:, 25:26],
                    in1=e[:],
                    op0=A.is_ge,
                    op1=A.mult,
                    accum_out=s,
                )
                nc.vector.reciprocal(r, s)
                ot = small.tile([2 * V, M], FP, tag="ot")
                nc.scalar.activation(ot[:], me[:], AF.Copy, scale=r)

                # ACT-queue DMA: same-engine in-order after ot, and keeps
                # the sync queue free for x-load issues.
                nc.scalar.dma_start(
                    out=out[2 * p : 2 * p + 2].rearrange("b v m -> (b v) m"),
                    in_=ot[:],
                )

            for n in range(nloc):
                chunked = n == 0 or n == last
                if n == 0:
                    xb = xb0
                else:
                    xb = xph.tile([128, 2 * T * V], FP, tag="xb")
                    if chunked:
                        # last batch: 4 contiguous (c-major) chunks, the
                        # final one only 0.5 MiB so the post-last-byte tail
                        # is one add+mm per half.  NOT more chunks: with 8
                        # the later issues fall back into the 8-sem DMA
                        # rotation window and chain to late completions
                        # (measured: last chunk issued at t=96us).  (A
                        # single strided dst AP covering both halves races:
                        # the write-range tracking misses the second h-run
                        # and the h1 adds can start early.)
                        for c in range(2):
                            lo, hi = (0, 3072) if c == 0 else (3072, 4096)
                            for h in range(2):
                                nc.sync.dma_start(
                                    out=xb[:, h * 4096 + lo : h * 4096 + hi],
                                    in_=x[n, h * 128 : (h + 1) * 128,
                                          lo // V : hi // V],
                                )
                    else:
                        nc.sync.dma_start(
                            out=xb[:].rearrange(
                                "p (h t v) -> p h t v", h=2, v=V
                            ),
                            in_=x[n].rearrange("(h c) t v -> c h t v", h=2),
                        )
                xh = [xb[:, 0 : T * V], xb[:, T * V : 2 * T * V]]

                if n % 2 == 0:
                    q2[n // 2] = small.tile(
                        [INTER, 2 * V], FP, tag="q2", name=f"q2_{n // 2}"
                    )

                # one DVE add level: t 64 -> 32 (a1 free = 32 t-slots x V),
                # then 8 accumulating matmuls fold the 32 slots into 8 PSUM
                # slots (tl, v) while contracting c.
                pa = psA.tile([INTER, 512], FP, tag="pa")
                mm_idx = 0
                if chunked:
                    # c-major: emit the adds (and their matmuls) in
                    # chunk-arrival order; bounds match the DMA chunking
                    # (batch 0 symmetric, last batch 3:1)
                    bounds = [0, 1024, 4096] if n == 0 else [0, 3072, 4096]
                    a1s = [
                        xp.tile([128, T * V // 2], FP, tag=f"a1{h}",
                                name=f"a1c{n}_{h}")
                        for h in range(2)
                    ]
                    for c in range(2):
                        for h in range(2):
                            # 512-grain add->mm ping-pong: a whole-chunk add
                            # (1.2us) would serialize ahead of both matmuls
                            # on the tail critical path
                            for j in range((bounds[c + 1] - bounds[c]) // 1024):
                                soff = bounds[c] + j * 1024
                                src = xh[h][
                                    :, soff : soff + 1024
                                ].rearrange(
                                    "p (t two v) -> p t two v", two=2, v=V
                                )
                                off = soff // 2
                                dst = a1s[h][
                                    :, off : off + 512
                                ].rearrange("p (t v) -> p t v", v=V)
                                nc.vector.tensor_add(
                                    dst, src[:, :, 0, :], src[:, :, 1, :]
                                )
                                mm = nc.tensor.matmul(
                                    pa[:],
                                    wq_sb[:, h, :],
                                    a1s[h][:, off : off + 512],
                                    start=(mm_idx == 0),
                                    stop=(mm_idx == 7),
                                )
                                if first_mm is None:
                                    first_mm = mm
                                    add_dep_helper(
                                        mm.ins, d2.ins, sync=False,
                                        reason="pe-wait-absorb order",
                                    )
                                mm_idx += 1
                else:
                    for h in range(2):
                        a1 = xp.tile([128, T * V // 2], FP, tag=f"a1{h}")
                        nc.vector.tensor_add(
                            a1[:],
                            xh[h][:, : T * V // 2],
                            xh[h][:, T * V // 2 :],
                        )
                        for j in range(4):
                            nc.tensor.matmul(
                                pa[:],
                                wq_sb[:, h, :],
                                a1[:, j * 512 : (j + 1) * 512],
                                start=(mm_idx == 0),
                                stop=(mm_idx == 7),
                            )
                            mm_idx += 1

                # Software pipelining: DVE executes in order, so anything
                # that waits on another engine must sit AFTER later batches'
                # adds or it stalls the add stream (and with it the DVE tick
                # counters that gate x-load DMA issue).  The reduce (waits on
                # the PE matmul group) runs one batch late; the score chain
                # (16 DVE ops waiting on the pb matmul) runs two late.
                pending.append((n, pa))
                if len(pending) > 1:
                    for sp in ready_scores:
                        finish_score(sp)
                    ready_scores = []
                    nr, par = pending.pop(0)
                    finish_reduce(nr, par)
                    if nr % 2 == 1:
                        ready_scores.append(nr // 2)

            for nr, par in pending:
                finish_reduce(nr, par)
                if nr % 2 == 1:
                    ready_scores.append(nr // 2)
            for sp in ready_scores:
                finish_score(sp)
    nc.finalize()
    return nc


_NC_CACHE: dict[int, bass.Bass] = {}


def _get_nc(nloc: int) -> bass.Bass:
    if nloc not in _NC_CACHE:
        _NC_CACHE[nloc] = build_nc(nloc)
    return _NC_CACHE[nloc]


def _make_in_maps(x, Wq, bq, key_prototypes, ncores):
    nloc = x.shape[0] // ncores
    wqt = np.ascontiguousarray(np.asarray(Wq, dtype=np.float32).T)
    kpc = np.ascontiguousarray(np.asarray(key_prototypes, dtype=np.float32))
    bqc = np.ascontiguousarray(
        np.asarray(bq, dtype=np.float32).reshape(INTER, 1)
    )
    xc = np.asarray(x, dtype=np.float32)
    return [
        {
            "x": np.ascontiguousarray(xc[i * nloc : (i + 1) * nloc]),
            "wqt": wqt,
            "kp": kpc,
            "bq": bqc,
        }
        for i in range(ncores)
    ]


def run(inputs, trace: bool = False):
    """Run on hardware; returns (full_output, BassKernelResults)."""
    from concourse.bass_utils import run_bass_kernel_spmd

    x = inputs["x"]
    nloc = x.shape[0] // NCORES
    nc = _get_nc(nloc)
    in_maps = _make_in_maps(
        x, inputs["Wq"], inputs["bq"], inputs["key_prototypes"], NCORES
    )
    res = run_bass_kernel_spmd(nc, in_maps, list(range(NCORES)), trace=trace)
    out = np.concatenate([r["out"] for r in res.results], axis=0)
    return out, res


def kernel(**inputs) -> np.ndarray:
    out, _ = run(inputs, trace=False)
    return out

